# revision 1
# baseline (speedup 1.0000x reference)
"""DIEN model Trainium2 kernel (8-core SPMD, batch-sharded).

Model (per reference): B=2048, S=200, D=H=ATT=64.
  1. Interest-extraction GRU over time.
  2. Concat-MLP attention + masked softmax over time.
  3. Attentional GRU (AGRU) scan -> final hidden (B, H).

Sharding: data-parallel on batch across 8 cores (256 rows/core),
weights replicated.

Structural facts used:
  * The pad mask never needs to be applied inside either scan: hidden
    states past a row's length only feed (a) attention scores that get
    -1e9 before softmax and (b) AGRU steps whose attention weight is
    exactly 0.0 (fp32 exp underflow), so h' = h exactly there.
  * All scan tensors are feature-major [feat, batch]; behavior is
    transposed on the fly with PE-transpose (2 time steps per 128x128
    transpose).
  * Attention runs batch-major fused into the GRU scan: the h state
    ping-pong buffers are [128, 256] with partitions 64..127 holding
    target^T, so lhsT = h_buf[:, bt] is concat([h_s, target])^T with
    zero extra ops.  A2 is folded into A1 on the host (rows scaled by
    |a2|, positives-first reorder) so the ATT contraction is a signed
    pair of free-axis tensor_reduces.
"""

import os
import numpy as np

B, S, D, H, ATT = 2048, 200, 64, 64, 64
NCORES = 8
BS = B // NCORES          # 256 batch rows per core
BT = BS // 128            # 2 batch tiles of 128

_CACHE = {}


def _build_program(npos):
    import concourse.bass as bass
    import concourse.mybir as mybir
    from concourse import bacc
    from concourse.tile import TileContext

    fp32 = mybir.dt.float32
    AF = mybir.ActivationFunctionType
    OP = mybir.AluOpType
    AX = mybir.AxisListType

    nc = bacc.Bacc(None, target_bir_lowering=False)

    # ---------------- DRAM I/O ----------------
    beh = nc.dram_tensor("behavior", [BS, S, D], fp32, kind="ExternalInput")
    tgt = nc.dram_tensor("target", [BS, D], fp32, kind="ExternalInput")
    lens = nc.dram_tensor("lengths_f", [BS, 1], fp32, kind="ExternalInput")
    wihT = nc.dram_tensor("wihT", [128, 3 * H], fp32, kind="ExternalInput")   # dup row halves
    whhT = nc.dram_tensor("whhT", [128, 3 * H], fp32, kind="ExternalInput")   # dup row halves
    a1sT = nc.dram_tensor("a1sT", [H + D, ATT], fp32, kind="ExternalInput")
    w4iT = nc.dram_tensor("w4iT", [H, 3 * H], fp32, kind="ExternalInput")     # [r|z|n] input parts
    w4hT = nc.dram_tensor("w4hT", [H, 3 * H], fp32, kind="ExternalInput")     # [r|z|n] hidden parts
    ident = nc.dram_tensor("ident", [128, 128], fp32, kind="ExternalInput")
    iota_r = nc.dram_tensor("iota_r", [1, S], fp32, kind="ExternalInput")
    ones_c = nc.dram_tensor("ones_c", [1, H], fp32, kind="ExternalInput")
    svec_d = nc.dram_tensor("svec", [128, 1], fp32, kind="ExternalInput")     # +1 x64, -1 x64
    bias2 = nc.dram_tensor("bias2", [128, 1], fp32, kind="ExternalInput")     # (bih+bhh)[r], -(..)[z]
    biasn = nc.dram_tensor("biasn", [128, 2], fp32, kind="ExternalInput")     # [0:64,0]=bih_n ; [64:128,1]=bhh_n
    bias4 = nc.dram_tensor("bias4", [128, 2], fp32, kind="ExternalInput")     # [:,0]=(br|bz) ; [0:64,1]=bn

    hout = nc.dram_tensor("h_out", [BS, H], fp32, kind="ExternalOutput")

    # DRAM scratch
    outs_d = nc.dram_tensor("outs_d", [S, H, BS], fp32)
    att_d = nc.dram_tensor("att_d", [S, BS], fp32)

    with TileContext(nc) as tc:
        with (
            tc.tile_pool(name="const", bufs=1) as cpool,
            tc.tile_pool(name="stage", bufs=6) as stage,
            tc.tile_pool(name="xt", bufs=6) as xtp,
            tc.tile_pool(name="hip", bufs=3) as hip,
            tc.tile_pool(name="ew", bufs=4) as ew,
            tc.tile_pool(name="relu", bufs=2) as relup,
            tc.tile_pool(name="ps2", bufs=1, space="PSUM") as ps2,   # prz{g}, pn{g}
            tc.tile_pool(name="ps1", bufs=1, space="PSUM") as ps1,   # phn{g}, pxt
            tc.tile_pool(name="ps3", bufs=2, space="PSUM") as ps3,   # p3
        ):
            # ---------------- constants into SBUF ----------------
            def cload(name, dram, shape):
                t = cpool.tile(shape, fp32, tag=name)
                nc.sync.dma_start(t[:], dram[:])
                return t

            wih_s = cload("wih", wihT, [128, 3 * H])
            whh_s = cload("whh", whhT, [128, 3 * H])
            a1_s = cload("a1", a1sT, [H + D, ATT])
            w4i_s = cload("w4i", w4iT, [H, 3 * H])
            w4h_s = cload("w4h", w4hT, [H, 3 * H])
            id_s = cload("id", ident, [128, 128])
            iota_s = cload("iota", iota_r, [1, S])
            ones_s = cload("ones", ones_c, [1, H])
            svec_s = cload("svec", svec_d, [128, 1])
            bias2_s = cload("bias2", bias2, [128, 1])
            biasn_s = cload("biasn", biasn, [128, 2])
            bias4_s = cload("bias4", bias4, [128, 2])
            lens_s = cpool.tile([128, BT], fp32, tag="lens")
            for bt in range(BT):
                nc.sync.dma_start(lens_s[:, bt : bt + 1], lens[bt * 128 : (bt + 1) * 128, :])

            # h ping-pong buffers [128, BS]: rows 0:64 = h_s, rows 64:128 = target^T
            hbuf = [cpool.tile([128, BS], fp32, tag=f"hbuf{i}", name=f"hbuf{i}") for i in range(2)]

            for bt in range(BT):
                tg_st = stage.tile([128, D], fp32, tag="tgst")
                nc.sync.dma_start(tg_st[:], tgt[bt * 128 : (bt + 1) * 128, :])
                pt = ps1.tile([128, 128], fp32, tag="phn0", name="pt")
                nc.tensor.transpose(pt[0:D, :], tg_st[:], id_s[:])
                tg_ev = stage.tile([D, 128], fp32, tag="tgev")
                nc.scalar.copy(tg_ev[:], pt[0:D, :])
                for i in range(2):
                    nc.gpsimd.tensor_copy(
                        out=hbuf[i][64:128, bt * 128 : (bt + 1) * 128], in_=tg_ev[:]
                    )
            nc.vector.memset(hbuf[0][0:64, :], 0.0)

            scores = [cpool.tile([128, S], fp32, tag=f"sc{bt}", name=f"sc{bt}") for bt in range(BT)]

            # =========== PHASE 2: GRU scan (+ fused attention MLP) ===========
            xt = None
            p3 = None
            for s in range(S):
                hp = hbuf[s % 2]          # h_{s-1} in rows 0:64
                hn_buf = hbuf[(s + 1) % 2]

                # ---- x^T for this step (2 steps per PE transpose) ----
                if s % 2 == 0:
                    xt = xtp.tile([128, BS], fp32, tag="xt")
                    for bt in range(BT):
                        bst = stage.tile([128, 128], fp32, tag="bst")
                        nc.sync.dma_start(
                            bst[:],
                            beh[bt * 128 : (bt + 1) * 128, s : s + 2, :].rearrange(
                                "b s d -> b (s d)"
                            ),
                        )
                        pxt = ps1.tile([128, 128], fp32, tag=f"phn{bt}", name="pxt")
                        nc.tensor.transpose(pxt[:], bst[:], id_s[:])
                        nc.scalar.copy(xt[:, bt * 128 : (bt + 1) * 128], pxt[:])
                half = s % 2
                x_s = xt[half * 64 : half * 64 + 64, :]
                tp_x = (half * 64, 0)
                wih_rows = wih_s[half * 64 : half * 64 + 64, :]

                # ---- gate pre-activations: two independent column groups ----
                for g in range(2):
                    cs = slice(g * 128, g * 128 + 128)
                    p_rz = ps2.tile([128, 128], fp32, tag=f"prz{g}", name=f"prz{g}")
                    nc.tensor.matmul(
                        p_rz[:], wih_rows[:, 0:128], x_s[:, cs],
                        start=True, stop=False, tile_position=tp_x,
                    )
                    nc.tensor.matmul(
                        p_rz[:], whh_s[0:64, 0:128], hp[0:64, cs],
                        start=False, stop=True, tile_position=(0, 0),
                    )
                    p_hn = ps1.tile([128, 128], fp32, tag=f"phn{g}", name=f"phn{g}")
                    nc.tensor.matmul(
                        p_hn[64:128, :], whh_s[0:64, 128:192], hp[0:64, cs],
                        start=True, stop=True, tile_position=(0, 64),
                    )
                    p_n = ps2.tile([H, 128], fp32, tag=f"pn{g}", name=f"pn{g}")
                    nc.tensor.matmul(
                        p_n[:], wih_rows[:, 128:192], x_s[:, cs],
                        start=True, stop=False, tile_position=tp_x,
                    )

                    rz = ew.tile([128, 128], fp32, tag=f"rz{g}", name=f"rz{g}")
                    nc.scalar.activation(rz[:], p_rz[:], AF.Sigmoid, bias=bias2_s[:], scale=svec_s[:])

                    t_t = ew.tile([128, 128], fp32, tag=f"tt{g}", name=f"tt{g}")
                    nc.vector.scalar_tensor_tensor(
                        t_t[64:128, :], p_hn[64:128, :], biasn_s[64:128, 1:2], rz[64:128, :],
                        op0=OP.add, op1=OP.mult,
                    )
                    if os.environ.get("DIEN_IAT", "pe") == "dve":
                        u_n = ew.tile([H, 128], fp32, tag=f"un{g}", name=f"un{g}")
                        nc.vector.tensor_tensor(u_n[:], t_t[64:128, :], p_n[:], OP.add)
                        n_t = ew.tile([H, 128], fp32, tag=f"nt{g}", name=f"nt{g}")
                        nc.scalar.activation(n_t[:], u_n[:], AF.Tanh, bias=biasn_s[0:64, 0:1])
                    else:
                        nc.tensor.matmul(
                            p_n[:], id_s[64:128, 64:128], t_t[64:128, :],
                            start=False, stop=True, tile_position=(64, 0),
                        )
                        n_t = ew.tile([H, 128], fp32, tag=f"nt{g}", name=f"nt{g}")
                        nc.scalar.activation(n_t[:], p_n[:], AF.Tanh, bias=biasn_s[0:64, 0:1])

                    d_t = ew.tile([H, 128], fp32, tag=f"dt{g}", name=f"dt{g}")
                    nc.vector.tensor_tensor(d_t[:], n_t[:], hp[0:64, cs], OP.subtract)
                    e_t = ew.tile([H, 128], fp32, tag=f"et{g}", name=f"et{g}")
                    nc.vector.tensor_tensor(e_t[:], d_t[:], rz[0:64, :], OP.mult)
                    nc.vector.tensor_tensor(hn_buf[0:64, cs], hp[0:64, cs], e_t[:], OP.add)

                    nc.sync.dma_start(outs_d[s, :, g * 128 : g * 128 + 128], hn_buf[0:64, cs])

                # ---- fused attention MLP ----
                slot = s % 8
                if slot == 0:
                    p3 = [ps3.tile([128, 512], fp32, tag="p3", name="p3") for _ in range(BT)]
                for bt in range(BT):
                    nc.tensor.matmul(
                        p3[bt][:, slot * 64 : slot * 64 + 64],
                        hn_buf[:, bt * 128 : (bt + 1) * 128], a1_s[:],
                        start=True, stop=True, tile_position=(0, 0),
                    )
                if slot == 7 or s == S - 1:
                    ns = slot + 1
                    base = s - slot
                    for bt in range(BT):
                        rb = relup.tile([128, 512], fp32, tag=f"rb{bt}")
                        nc.scalar.activation(rb[:, 0 : ns * 64], p3[bt][:, 0 : ns * 64], AF.Relu)
                        rbv = rb[:].rearrange("p (t a) -> p t a", a=64)
                        pos = relup.tile([128, 8], fp32, tag=f"pos{bt}")
                        nc.vector.tensor_reduce(
                            pos[:, 0:ns], rbv[:, 0:ns, 0:npos], axis=AX.X, op=OP.add
                        )
                        neg = relup.tile([128, 8], fp32, tag=f"neg{bt}")
                        nc.vector.tensor_reduce(
                            neg[:, 0:ns], rbv[:, 0:ns, npos:64], axis=AX.X, op=OP.add
                        )
                        nc.vector.tensor_tensor(
                            scores[bt][:, base : base + ns], pos[:, 0:ns], neg[:, 0:ns],
                            OP.subtract,
                        )

            # =========== PHASE 3 tail: mask + softmax + att^T to DRAM ===========
            for bt in range(BT):
                iob = ew.tile([128, S], fp32, tag="iob")
                nc.gpsimd.partition_broadcast(iob[:], iota_s[0:1, :])
                negb = ew.tile([128, S], fp32, tag="negb")
                nc.vector.memset(negb[:], -1e9)
                pen = ew.tile([128, S], fp32, tag="pen")
                nc.vector.scalar_tensor_tensor(
                    pen[:], iob[:], lens_s[:, bt : bt + 1], negb[:],
                    op0=OP.is_ge, op1=OP.mult,
                )
                nc.vector.tensor_tensor(scores[bt][:], scores[bt][:], pen[:], OP.add)
                mx = ew.tile([128, 1], fp32, tag="mx")
                nc.vector.tensor_reduce(mx[:], scores[bt][:], axis=AX.X, op=OP.max, negate=True)
                ex = ew.tile([128, S], fp32, tag="ex")
                sm = ew.tile([128, 1], fp32, tag="sm")
                nc.scalar.activation(ex[:], scores[bt][:], AF.Exp, bias=mx[:], accum_out=sm[:])
                rcp = ew.tile([128, 1], fp32, tag="rcp")
                nc.vector.reciprocal(rcp[:], sm[:])
                aw = ew.tile([128, S], fp32, tag="aw")
                nc.vector.tensor_scalar_mul(aw[:], ex[:], rcp[:])
                for c0, cn in ((0, 128), (128, S - 128)):
                    pat = ps1.tile([128, 128], fp32, tag="phn0", name="pat")
                    nc.tensor.transpose(pat[0:cn, :], aw[:, c0 : c0 + cn], id_s[:])
                    sat = stage.tile([128, 128], fp32, tag="sat")
                    nc.scalar.copy(sat[0:cn, :], pat[0:cn, :])
                    nc.sync.dma_start(
                        att_d[c0 : c0 + cn, bt * 128 : (bt + 1) * 128], sat[0:cn, :]
                    )

            # =========== PHASE 4: attentional GRU scan ===========
            h4 = [cpool.tile([H, BS], fp32, tag=f"h4_{i}", name=f"h4_{i}") for i in range(2)]
            nc.vector.memset(h4[0][:], 0.0)
            hi = None
            ar = None
            for s in range(S):
                hp4 = h4[s % 2]
                hn4 = h4[(s + 1) % 2]

                if s % 8 == 0:
                    ns = min(8, S - s)
                    hi = hip.tile([H, 8 * BS], fp32, tag="hi")
                    nc.sync.dma_start(
                        hi[:, 0 : ns * BS].rearrange("h (s b) -> h s b", b=BS),
                        outs_d[s : s + ns, :, :].rearrange("s h b -> h s b"),
                    )
                    ar = hip.tile([1, 8 * BS], fp32, tag="ar")
                    nc.sync.dma_start(
                        ar[:, 0 : ns * BS].rearrange("o (s b) -> o s b", b=BS),
                        att_d[s : s + ns, :].rearrange("(o s) b -> o s b", o=1),
                    )
                hi_s = hi[:, (s % 8) * BS : (s % 8) * BS + BS]
                a_row = ar[:, (s % 8) * BS : (s % 8) * BS + BS]

                for g in range(2):
                    cs = slice(g * 128, g * 128 + 128)
                    p_rz = ps2.tile([128, 128], fp32, tag=f"prz{g}", name=f"prz{g}")
                    nc.tensor.matmul(
                        p_rz[:], w4i_s[:, 0:128], hi_s[:, cs], start=True, stop=False,
                        tile_position=(0, 0),
                    )
                    nc.tensor.matmul(
                        p_rz[:], w4h_s[:, 0:128], hp4[:, cs], start=False, stop=True,
                        tile_position=(0, 0),
                    )
                    rz = ew.tile([128, 128], fp32, tag=f"rz{g}", name=f"rz{g}")
                    nc.scalar.activation(rz[:], p_rz[:], AF.Sigmoid, bias=bias4_s[:, 0:1])

                    # attention-weight broadcast [H, 128] on GpSimd
                    a_b = ew.tile([H, 128], fp32, tag=f"ab{g}", name=f"ab{g}")
                    nc.gpsimd.partition_broadcast(a_b[:], a_row[:, cs])
                    rh = ew.tile([H, 128], fp32, tag=f"rh{g}", name=f"rh{g}")
                    nc.vector.tensor_tensor(rh[:], rz[0:64, :], hp4[:, cs], OP.mult)
                    p_n = ps2.tile([H, 128], fp32, tag=f"pn{g}", name=f"pn{g}")
                    nc.tensor.matmul(
                        p_n[:], w4i_s[:, 128:192], hi_s[:, cs], start=True, stop=False,
                        tile_position=(0, 0),
                    )
                    nc.tensor.matmul(
                        p_n[:], w4h_s[:, 128:192], rh[:], start=False, stop=True,
                        tile_position=(0, 0),
                    )
                    n_t = ew.tile([H, 128], fp32, tag=f"nt{g}", name=f"nt{g}")
                    nc.scalar.activation(n_t[:], p_n[:], AF.Tanh, bias=bias4_s[0:64, 1:2])

                    zs = ew.tile([H, 128], fp32, tag=f"zs{g}", name=f"zs{g}")
                    nc.gpsimd.tensor_copy(out=zs[:], in_=rz[64:128, :])
                    zp = ew.tile([H, 128], fp32, tag=f"zp{g}", name=f"zp{g}")
                    nc.vector.tensor_tensor(zp[:], zs[:], a_b[:], OP.mult)
                    d_t = ew.tile([H, 128], fp32, tag=f"dt{g}", name=f"dt{g}")
                    nc.vector.tensor_tensor(d_t[:], n_t[:], hp4[:, cs], OP.subtract)
                    e_t = ew.tile([H, 128], fp32, tag=f"et{g}", name=f"et{g}")
                    nc.vector.tensor_tensor(e_t[:], d_t[:], zp[:], OP.mult)
                    nc.vector.tensor_tensor(hn4[:, cs], hp4[:, cs], e_t[:], OP.add)

            # =========== epilogue: h4 -> [BS, H] -> DRAM ===========
            hfin = h4[S % 2]
            for bt in range(BT):
                pf = ps1.tile([128, 128], fp32, tag="phn0", name="pf")
                nc.tensor.transpose(pf[:, 0:H], hfin[:, bt * 128 : (bt + 1) * 128], id_s[0:H, 0:H])
                sf = stage.tile([128, H], fp32, tag="sf")
                nc.scalar.copy(sf[:], pf[:, 0:H])
                nc.sync.dma_start(hout[bt * 128 : (bt + 1) * 128, :], sf[:])

    nc.finalize()
    return nc


def _prep_host_inputs(inputs):
    behavior = np.ascontiguousarray(np.asarray(inputs["behavior"], dtype=np.float32))
    target = np.ascontiguousarray(np.asarray(inputs["target"], dtype=np.float32))
    lengths = np.asarray(inputs["lengths"]).astype(np.float32).reshape(B, 1)
    Wih = np.asarray(inputs["Wih"], dtype=np.float32)
    Whh = np.asarray(inputs["Whh"], dtype=np.float32)
    bih = np.asarray(inputs["bih"], dtype=np.float32)
    bhh = np.asarray(inputs["bhh"], dtype=np.float32)
    A1 = np.asarray(inputs["A1"], dtype=np.float32)
    b1 = np.asarray(inputs["b1"], dtype=np.float32)
    A2 = np.asarray(inputs["A2"], dtype=np.float32).reshape(-1)
    Wr = np.asarray(inputs["Wr"], dtype=np.float32)
    Wz = np.asarray(inputs["Wz"], dtype=np.float32)
    Wn = np.asarray(inputs["Wn"], dtype=np.float32)
    br = np.asarray(inputs["br"], dtype=np.float32)
    bz = np.asarray(inputs["bz"], dtype=np.float32)
    bn = np.asarray(inputs["bn"], dtype=np.float32)

    assert not np.any(b1), "nonzero b1 not supported by this kernel build"

    # ph2 gate column order [z | r | n]: z's sigma output lands on
    # partitions 0:64 (used by the h-update), r on 64:128 (used by the
    # n-gate path which lives on partitions 64:128).
    perm = np.concatenate([np.arange(64, 128), np.arange(0, 64), np.arange(128, 192)])
    wihT = np.concatenate([Wih.T[:, perm], Wih.T[:, perm]], axis=0).astype(np.float32)
    whhT = np.concatenate([Whh.T[:, perm], Whh.T[:, perm]], axis=0).astype(np.float32)

    order = np.argsort(~(A2 > 0), kind="stable")
    npos = int((A2 > 0).sum())
    A1s = (np.abs(A2)[:, None] * A1)[order]
    a1sT = np.ascontiguousarray(A1s.T)

    w4iT = np.concatenate([Wr[:, 0:H].T, Wz[:, 0:H].T, Wn[:, 0:H].T], axis=1)
    w4hT = np.concatenate([Wr[:, H:].T, Wz[:, H:].T, Wn[:, H:].T], axis=1)

    ident = np.eye(128, dtype=np.float32)
    iota_r = np.arange(S, dtype=np.float32).reshape(1, S)
    ones_c = np.ones((1, H), np.float32)
    # sigma arg = svec*u + bias2 ; rows 0:64 are z (negated -> 1-z), rows 64:128 are r
    svec = np.concatenate([-np.ones(64, np.float32), np.ones(64, np.float32)]).reshape(128, 1)
    g2 = bih[0:128] + bhh[0:128]
    bias2 = np.concatenate([-g2[64:128], g2[0:64]]).reshape(128, 1).astype(np.float32)
    biasn = np.zeros((128, 2), np.float32)
    biasn[0:64, 0] = bih[128:192]
    biasn[64:128, 1] = bhh[128:192]
    bias4 = np.zeros((128, 2), np.float32)
    bias4[0:64, 0] = br
    bias4[64:128, 0] = bz
    bias4[0:64, 1] = bn

    shared = dict(
        wihT=wihT, whhT=whhT, a1sT=a1sT,
        w4iT=np.ascontiguousarray(w4iT), w4hT=np.ascontiguousarray(w4hT),
        ident=ident, iota_r=iota_r, ones_c=ones_c, svec=svec,
        bias2=bias2, biasn=biasn, bias4=bias4,
    )
    in_maps = []
    for c in range(NCORES):
        sl = slice(c * BS, (c + 1) * BS)
        m = dict(shared)
        m["behavior"] = np.ascontiguousarray(behavior[sl])
        m["target"] = np.ascontiguousarray(target[sl])
        m["lengths_f"] = np.ascontiguousarray(lengths[sl])
        in_maps.append(m)
    return in_maps, npos


def kernel(**inputs) -> np.ndarray:
    from concourse.bass_utils import run_bass_kernel_spmd

    in_maps, npos = _prep_host_inputs(inputs)
    if npos not in _CACHE:
        _CACHE[npos] = _build_program(npos)
    nc = _CACHE[npos]

    trace = os.environ.get("DIEN_TRACE", "0") == "1"
    res = run_bass_kernel_spmd(nc, in_maps, core_ids=list(range(NCORES)), trace=trace)
    out = np.concatenate([r["h_out"] for r in res.results], axis=0)
    kernel._last_exec_time_ns = res.exec_time_ns
    return out.astype(np.float32)

